# revision 1
# baseline (speedup 1.0000x reference)
"""Trainium2 Bass kernel for the HFNN (hierarchical fuzzy NN) forward pass.

Math (per branch k of 8, rule r of 32, feature f of 16, batch b of 32768):
  expo[k,b,r]  = sum_f (x-mu)^2 / (2 sigma^2)
  E            = exp(-expo);  normalized over r
  conq[k,b,r]  = w3_bias + sum_f w3 * x
  tsk[k,b]     = sum_r E*conq / sum_r E
  out          = softmax(w5 @ tsk + b5) over 2 classes

Device strategy (pure batch data-parallel over 8 cores, 4096 batch each):
  - Host ships per core ONE fp32r slab x [128, 4096] (row 16k+f = feature f
    of branch k); GPSIMD squares each 512-col chunk on-device (x2 tile).
  - Per chunk: fp32r membership matmuls as K=64 row-tile pairs — quad on
    x2[0:64]/x2[64:128] (tile positions (0,0)/(64,0)), linear term
    accumulating on x[0:64]/x[64:128]; group-1 weights live at SBUF
    partitions 64-127 so base-partition auto-derivation is legal. ACT
    computes E = exp(m + bias) per chunk (fp32r); conq0 = w3.x via the same
    K=64 pairing; DVE computes EC = E * conq0.
  - fp32r reduction matmuls with host-padded M=128 weights accumulate
    den = sum_r E, numb = sum_r E*w3bias, num0 = sum_r EC for 2 chunks into
    one PSUM collector bank (rows 24q+{0-7,8-15,16-23}); ACT/DVE copy it to
    SBUF; contiguous [48, 512] DMAs ship only the used rows.
  - Host does the remaining O(B) work exactly in float64: num = num0 + numb,
    tsk = num/den, d = (w5[0]-w5[1]).tsk + (b5[0]-b5[1]), p = sigmoid(+-d).
"""

import numpy as np

import concourse.bacc as bacc
import concourse.tile as tile
from concourse import mybir
from concourse.bass_utils import run_bass_kernel_spmd

F32 = mybir.dt.float32
F32R = mybir.dt.float32r

NB, NR, NF = 8, 32, 16
NBATCH, NCORE = 32768, 8
BC = NBATCH // NCORE          # 4096 batch per core
CH = 512                      # chunk (psum bank) width
NCH = BC // CH                # 8 chunks
NROUND = 4                    # collector rounds
CPR = NCH // NROUND           # chunks per round = 4
NSC = NCH // 2                # superchunks (1024 wide) = 4

_CACHE: dict = {}


def _build_nc():
    nc = bacc.Bacc("TRN2", target_bir_lowering=False, debug=False)
    x_in = nc.dram_tensor("x", [128, BC], F32R, kind="ExternalInput")
    # main weights: quad(g0|g1 stacked) | lin | conq, one 128-col block each
    wall_in = nc.dram_tensor("wall", [128, 512], F32R, kind="ExternalInput")
    # padded reduction weights: 8 x [128, 128], idx = 4*g + 2*q + kind
    wpad_in = nc.dram_tensor("wpad", [128, 1024], F32R, kind="ExternalInput")
    bias_in = nc.dram_tensor("ebias", [128, 2], F32, kind="ExternalInput")
    out_c = nc.dram_tensor("outc", [NROUND, 48, CH], F32, kind="ExternalOutput")

    with tile.TileContext(nc) as tc:
        with (
            tc.tile_pool(name="wpool", bufs=1) as wpool,
            tc.tile_pool(name="spool", bufs=16) as spool,
            tc.tile_pool(name="epool", bufs=12) as epool,
            tc.tile_pool(name="opool", bufs=4) as opool,
            tc.tile_pool(name="mps", bufs=2, space="PSUM") as mps,
            tc.tile_pool(name="cps", bufs=4, space="PSUM") as cps,
            tc.tile_pool(name="collps", bufs=2, space="PSUM") as collps,
        ):
            wall = wpool.tile([128, 512], F32R, tag="wall")
            nc.gpsimd.dma_start(out=wall[:], in_=wall_in[:, :])
            wpad = wpool.tile([128, 1024], F32R, tag="wpad")
            bias_t = wpool.tile([128, 2], F32, tag="bias")
            nc.gpsimd.dma_start(out=bias_t[:], in_=bias_in[:, :])

            warm = wpool.tile([128, 1], F32, tag="warm")
            nc.vector.memset(warm[:], 0.0)
            nc.scalar.activation(
                warm[:], warm[:], mybir.ActivationFunctionType.Exp
            )

            def wquad(g):
                return wall[64 * g : 64 * g + 64, 0:128]

            def wlin(g):
                return wall[64 * g : 64 * g + 64, 128:256]

            def wG(g):
                return wall[:, 256 + 128 * g : 384 + 128 * g]

            def wred(g, q, kind):
                i = 4 * g + 2 * q + kind
                return wpad[:, 128 * i : 128 * (i + 1)]

            for sc in range(NSC):
                rnd = sc
                coll = collps.tile([128, CH], F32, tag="coll", name=f"coll{rnd}")
                s_t = []
                e_t = [
                    epool.tile([128, 1024], F32R, tag="e", name=f"e{sc}{gg}")
                    for gg in range(2)
                ]
                for h in range(2):
                    j = 2 * sc + h
                    st = spool.tile([128, CH], F32R, tag="s")
                    nc.sync.dma_start(
                        out=st[:], in_=x_in[:, j * CH : (j + 1) * CH]
                    )
                    s_t.append(st)
                    x2 = spool.tile([128, CH], F32R, tag="x2", name=f"x2{j}")
                    if j == 0:
                        nc.vector.tensor_mul(x2[:], st[:], st[:])
                        nc.gpsimd.dma_start(out=wpad[:], in_=wpad_in[:, :])
                    else:
                        nc.gpsimd.tensor_mul(x2[:], st[:], st[:])
                    for g in range(2):
                        mt = mps.tile([128, CH], F32, tag="m", name=f"m{j}{g}")
                        nc.tensor.matmul(
                            mt[:], wquad(g), x2[64 * g : 64 * g + 64, :],
                            start=True, stop=False,
                        )
                        nc.tensor.matmul(
                            mt[:], wlin(g), st[64 * g : 64 * g + 64, :],
                            start=False, stop=True,
                        )
                        nc.scalar.activation(
                            e_t[g][:, h * CH : (h + 1) * CH], mt[:],
                            mybir.ActivationFunctionType.Exp,
                            bias=bias_t[:, g : g + 1], scale=1.0,
                        )
                for h in range(2):
                    j = 2 * sc + h
                    q = h
                    first = (q == 0)
                    g_ps = cps.tile([128, CH], F32, tag="c", name=f"g{j}")
                    for g in range(2):
                        nc.tensor.matmul(
                            g_ps[:], wG(g),
                            e_t[g][:, h * CH : (h + 1) * CH],
                            start=(g == 0), stop=(g == 1),
                        )
                    xg = epool.tile([128, CH], F32R, tag="xg", name=f"xg{j}")
                    nc.vector.tensor_mul(xg[:], s_t[h][:], g_ps[:])
                    for g in range(2):
                        nc.tensor.matmul(
                            coll[:], wred(g, q, 0),
                            e_t[g][:, h * CH : (h + 1) * CH],
                            start=(first and g == 0), stop=False,
                        )
                    nc.tensor.matmul(
                        coll[:], wred(0, q, 1), xg[:],
                        start=False, stop=(q == 1),
                    )
                ot = opool.tile([128, CH], F32, tag="o")
                if sc % 2 == 0:
                    nc.scalar.copy(ot[:48], coll[:48])
                else:
                    nc.vector.tensor_copy(ot[:48], coll[:48])
                nc.scalar.dma_start(out=out_c[rnd], in_=ot[:48])
    nc.finalize()
    return nc


def _host_prep(data, para_mu, para_sigma, para_w3):
    xt = np.ascontiguousarray(data.transpose(0, 2, 1)).astype(np.float32)
    xslab = xt.reshape(128, NBATCH)

    sig2 = para_sigma.astype(np.float64) ** 2
    mu = para_mu.astype(np.float64)
    a_neg = -1.0 / (2.0 * sig2)                     # [8, 32, 16]
    m2 = mu / sig2
    c = np.sum(mu * mu / (2.0 * sig2), axis=-1)     # [8, 32]

    wall = np.zeros((128, 512), np.float32)
    ebias = np.zeros((128, 2), np.float32)
    for g in range(2):
        for i in range(4):
            k = 4 * g + i
            rows = slice(64 * g + 16 * i, 64 * g + 16 * i + 16)
            cols = slice(32 * i, 32 * i + 32)
            wall[rows, cols] = a_neg[k].T
            wall[rows, 128 + 32 * i : 128 + 32 * i + 32] = m2[k].T
            # G weights: lhsT[32i + r, 16k + f] = w3[k, r, f]
            wall[32 * i : 32 * i + 32,
                 256 + 128 * g + 16 * k : 256 + 128 * g + 16 * k + 16] = (
                para_w3[k, :, :NF]
            )
            ebias[32 * i : 32 * i + 32, g] = -c[k]

    wpad = np.zeros((128, 1024), np.float32)
    for g in range(2):
        for q in range(2):
            for i in range(4):
                k = 4 * g + i
                rows = slice(32 * i, 32 * i + 32)
                c_red = 128 * (4 * g + 2 * q)
                c_num = 128 * (4 * g + 2 * q + 1)
                wpad[rows, c_red + 24 * q + k] = 1.0
                wpad[rows, c_red + 24 * q + 8 + k] = para_w3[k, :, NF]
    for q in range(2):
        c_num = 128 * (2 * q + 1)
        for k in range(NB):
            wpad[16 * k : 16 * k + 16, c_num + 24 * q + 16 + k] = 1.0
    return xslab, wall, wpad, ebias


def kernel(data, para_mu, para_sigma, para_w3, w5, b5):
    if "nc" not in _CACHE:
        _CACHE["nc"] = _build_nc()
    nc = _CACHE["nc"]

    xslab, wall, wpad, ebias = _host_prep(data, para_mu, para_sigma, para_w3)
    in_maps = []
    for cidx in range(NCORE):
        cols = slice(cidx * BC, (cidx + 1) * BC)
        in_maps.append(
            {
                "x": np.ascontiguousarray(xslab[:, cols]),
                "wall": wall,
                "wpad": wpad,
                "ebias": ebias,
            }
        )
    try:
        res = run_bass_kernel_spmd(nc, in_maps, core_ids=list(range(NCORE)))
    except Exception:
        # transient NRT device errors (e.g. a wedged core) recover on retry
        res = run_bass_kernel_spmd(nc, in_maps, core_ids=list(range(NCORE)))
    _CACHE["last_result"] = res

    # ---- host epilogue (exact, O(B)) ----
    den = np.empty((NB, NBATCH), np.float64)
    numb = np.empty((NB, NBATCH), np.float64)
    num0 = np.empty((NB, NBATCH), np.float64)
    for cidx in range(NCORE):
        arr = res.results[cidx]["outc"].astype(np.float64)  # [4, 48, 512]
        v = np.moveaxis(arr.reshape(NROUND, 2, 24, CH), 2, 0)
        v = v.reshape(24, BC)  # row l, local batch (rnd, q, t)
        cols = slice(cidx * BC, (cidx + 1) * BC)
        den[:, cols] = v[0:8]
        numb[:, cols] = v[8:16]
        num0[:, cols] = v[16:24]

    tsk = (num0 + numb) / den                     # [8, B]
    w5d = (w5[0] - w5[1]).astype(np.float64)
    d = w5d @ tsk + (float(b5[0]) - float(b5[1]))
    p0 = 1.0 / (1.0 + np.exp(-d))
    out = np.empty((NBATCH, 2), np.float32)
    out[:, 0] = p0.astype(np.float32)
    out[:, 1] = (1.0 - p0).astype(np.float32)
    return out

